# revision 10
# baseline (speedup 1.0000x reference)
"""Trainium2 Bass kernel for the GNN message-passing module (SAGE-GRU style).

Strategy (8 NeuronCores, SPMD single program):
  - Nodes sharded 8 ways (6250/core, padded to 6272 = 49*128).
  - Edges partitioned by destination-node owner, sorted by dst tile, slotted
    into 128-edge chunks (M chunks per 128-dst output tile, uniform across
    cores so the single SPMD program works for every core).
  - mean-aggregation = indirect-DMA row gather (node-major) + TensorE matmul
    against an on-device-built one-hot selection matrix, accumulated in PSUM,
    scaled by 1/deg at the ScalarE drain.
  - Dense 512x512 matmuls run feature-major in fp16 (full PE rate), fp32 PSUM
    accumulation over K=2048 groups, fused bias+tanh/sigmoid drains.
  - h_N and r*h_N are AllGathered (fp16) so the later aggregations stay local.

Dead code in the reference is eliminated: `_x_transformed` is unused, and
r_v = (gamma+1)*0 + beta = beta, so h_N_new = h + beta.
"""

import numpy as np

# ---------------------------------------------------------------- constants
N, E, D = 50000, 400000, 512
W = 8                     # cores
NPC = N // W              # 6250 nodes per core
NT = 49                   # dst tiles per core
NPAD = NT * 128           # 6272
NTAB = W * NPAD           # 50176 rows in padded tables
KT = 4                    # K tiles (512 / 128)
FT = 4                    # out-feature tiles

# weight order in the packed weight tensor; each entry is (param, key)
_W_ORDER = [
    ("sage_hidden", "Wl"), ("sage_hidden", "Wr"),
    ("W_beta1", "W"), ("W_beta2", "W"),
    ("sage_single_x", "Wl"), ("sage_single_x", "Wr"),
    ("sage_single_h", "Wl"), ("sage_single_h", "Wr"),
    ("sage_update_x", "Wl"), ("sage_update_x", "Wr"),
    ("sage_update_h", "Wl"), ("sage_update_h", "Wr"),
    ("sage_candidate_x", "Wl"), ("sage_candidate_x", "Wr"),
    ("sage_candidate_h", "Wl"), ("sage_candidate_h", "Wr"),
]
(W_HID_WL, W_HID_WR, W_B1, W_B2, W_SX_WL, W_SX_WR, W_SH_WL, W_SH_WR,
 W_UX_WL, W_UX_WR, W_UH_WL, W_UH_WR, W_CX_WL, W_CX_WR, W_CH_WL, W_CH_WR) = range(16)
NW = 16

B_HNO, B_BETA, B_R, B_Z, B_HT = range(5)   # combined-bias indices

SENTINEL = 300.0  # dstloc value for padded slots (never matches iota 0..127)


def _pack_wT(Wmat):
    """[dout, din] weight -> [128, KT*512] fp16 of W.T k-tiles (lhsT layout)."""
    WT = np.ascontiguousarray(Wmat.T.astype(np.float32))  # [din(k), dout(f)]
    blocks = [WT[k * 128:(k + 1) * 128, :] for k in range(KT)]
    return np.concatenate(blocks, axis=1).astype(np.float16)  # [128, 2048]


def _host_prep(x, h, edge_index, params):
    x = np.asarray(x, np.float32)
    h = np.asarray(h, np.float32)
    src = np.asarray(edge_index[0], np.int64)
    dst = np.asarray(edge_index[1], np.int64)

    deg = np.bincount(dst, minlength=N).astype(np.float32)
    inv_deg = 1.0 / np.maximum(deg, 1.0)

    order = np.argsort(dst, kind="stable")
    src_s, dst_s = src[order], dst[order]

    # global tile id: core c tile t  ->  c*NT + t
    core_s = dst_s // NPC
    loc_s = dst_s - core_s * NPC
    tile_s = core_s * NT + loc_s // 128
    # per-(core,tile) counts; edges for a tile are contiguous in sorted order
    cnt = np.bincount(tile_s, minlength=W * NT)
    starts = np.concatenate([[0], np.cumsum(cnt)])
    M = int(np.max((cnt + 127) // 128))
    C = NT * M

    # padded-table row id of every source node
    src_row = (src_s // NPC) * NPAD + (src_s % NPC)
    dstloc = (loc_s % 128).astype(np.float32)

    idx_T = np.zeros((W, 128, C), np.int32)
    dstloc_T = np.full((W, 128, C), SENTINEL, np.float16)
    for c in range(W):
        for t in range(NT):
            g = c * NT + t
            s0, n = starts[g], cnt[g]
            if n == 0:
                continue
            rows = src_row[s0:s0 + n]
            locs = dstloc[s0:s0 + n]
            o2 = np.argsort(rows, kind="stable")  # src-sorted within tile
            rows, locs = rows[o2], locs[o2]
            nc_ = int(n)
            col0 = t * M
            full = np.zeros(M * 128, np.int32)
            full[:nc_] = rows
            floc = np.full(M * 128, SENTINEL, np.float32)
            floc[:nc_] = locs
            idx_T[c, :, col0:col0 + M] = full.reshape(M, 128).T
            dstloc_T[c, :, col0:col0 + M] = floc.reshape(M, 128).T.astype(np.float16)

    # per-core node-sharded tensors
    x_fm = np.zeros((W, D, NPAD), np.float16)
    h_fm = np.zeros((W, D, NPAD), np.float16)
    invdeg_T = np.ones((W, 128, NT), np.float32)
    for c in range(W):
        xs = x[c * NPC:(c + 1) * NPC]
        hs = h[c * NPC:(c + 1) * NPC]
        x_fm[c, :, :NPC] = xs.T.astype(np.float16)
        h_fm[c, :, :NPC] = hs.T.astype(np.float16)
        iv = np.ones(NPAD, np.float32)
        iv[:NPC] = inv_deg[c * NPC:(c + 1) * NPC]
        invdeg_T[c] = iv.reshape(NT, 128).T

    # replicated gather table [NTAB, 2*D] fp16 : row = x[i] ++ h[i]
    xh_tab = np.zeros((NTAB, 2 * D), np.float16)
    for c in range(W):
        xh_tab[c * NPAD:c * NPAD + NPC, :D] = x[c * NPC:(c + 1) * NPC].astype(np.float16)
        xh_tab[c * NPAD:c * NPAD + NPC, D:] = h[c * NPC:(c + 1) * NPC].astype(np.float16)

    # weights + biases
    wpack = np.concatenate(
        [_pack_wT(np.asarray(params[p][k], np.float32)) for p, k in _W_ORDER], axis=1
    )  # [128, NW*2048] fp16

    b_hno = np.asarray(params["sage_hidden"]["bl"], np.float32)
    b_beta = (np.asarray(params["W_beta1"]["b"], np.float32)
              + np.asarray(params["W_beta2"]["b"], np.float32))
    b_r = (np.asarray(params["sage_single_x"]["bl"], np.float32)
           + np.asarray(params["sage_single_h"]["bl"], np.float32))
    b_z = (np.asarray(params["sage_update_x"]["bl"], np.float32)
           + np.asarray(params["sage_update_h"]["bl"], np.float32))
    b_ht = (np.asarray(params["sage_candidate_x"]["bl"], np.float32)
            + np.asarray(params["sage_candidate_h"]["bl"], np.float32))
    biases = np.stack([b.reshape(FT, 128).T for b in [b_hno, b_beta, b_r, b_z, b_ht]],
                      axis=1).reshape(128, 5 * FT, order="F")
    # biases[:, bi*FT + f] = b_bi[f*128:(f+1)*128]  -- build explicitly to be safe
    biases = np.zeros((128, 5 * FT), np.float32)
    for bi, b in enumerate([b_hno, b_beta, b_r, b_z, b_ht]):
        for f in range(FT):
            biases[:, bi * FT + f] = b[f * 128:(f + 1) * 128]

    iota_row = np.tile(np.arange(128, dtype=np.float16)[None, :], (128, 1))
    eye16 = np.eye(128, dtype=np.float16)
    eye32 = np.eye(128, dtype=np.float32)
    c16 = np.concatenate([iota_row, eye16], axis=1)              # [128, 256]
    c32 = np.stack([invdeg_T[c] for c in range(W)])              # per-core part
    c32 = np.concatenate(
        [np.tile(eye32[None], (W, 1, 1)),
         c32,
         np.tile(biases[None], (W, 1, 1))], axis=2)              # [W, 128, 128+NT+20]

    return dict(M=M, C=C, idx_T=idx_T, dstloc_T=dstloc_T, x_fm=x_fm, h_fm=h_fm,
                xh_tab=xh_tab, wpack=wpack, c16=c16, c32=c32)


# ---------------------------------------------------------------- program
_PROGRAM_CACHE = {}

# node chunks for the dense phases
_CHUNKS = [(i * 512, 512) for i in range(12)] + [(6144, 128)]


def _build_program(M, dbg=False):
    import concourse.bass as bass
    import concourse.mybir as mybir
    import concourse.tile as tile
    from concourse import bacc
    from concourse.bass import IndirectOffsetOnAxis

    f16, f32, i32 = mybir.dt.float16, mybir.dt.float32, mybir.dt.int32
    AF = mybir.ActivationFunctionType
    ALU = mybir.AluOpType
    C = NT * M
    RG = [list(range(W))]

    nc = bacc.Bacc("TRN2", target_bir_lowering=False, debug=False,
                   enable_asserts=False, num_devices=W)

    # ---- I/O
    t_xfm = nc.dram_tensor("x_fm", [D, NPAD], f16, kind="ExternalInput")
    t_hfm = nc.dram_tensor("h_fm", [D, NPAD], f16, kind="ExternalInput")
    t_xh = nc.dram_tensor("xh_tab", [NTAB, 2 * D], f16, kind="ExternalInput")
    t_idx = nc.dram_tensor("idx_t", [128, C], i32, kind="ExternalInput")
    t_dl = nc.dram_tensor("dstloc_t", [128, C], f16, kind="ExternalInput")
    t_w = nc.dram_tensor("wpack", [128, NW * KT * 512], f16, kind="ExternalInput")
    t_c16 = nc.dram_tensor("c16", [128, 256], f16, kind="ExternalInput")
    t_c32 = nc.dram_tensor("c32", [128, 128 + NT + 20], f32, kind="ExternalInput")
    t_out = nc.dram_tensor("out", [NPAD, D], f32, kind="ExternalOutput")

    # ---- internal DRAM
    ag1in = nc.dram_tensor("ag1in", [NPAD, D], f16, kind="Internal")
    ag1out = nc.dram_tensor("ag1out", [NTAB, D], f16, kind="Internal")
    ag2in = nc.dram_tensor("ag2in", [NPAD, D], f16, kind="Internal")
    ag2out = nc.dram_tensor("ag2out", [NTAB, D], f16, kind="Internal")
    _dk = "ExternalOutput" if dbg else "Internal"
    xagg_hbm = nc.dram_tensor("xagg_fm", [D, NPAD], f16, kind=_dk)
    rh_hbm = nc.dram_tensor("rh_fm", [D, NPAD], f16, kind=_dk)
    z_hbm = nc.dram_tensor("z_fm", [D, NPAD], f16, kind=_dk)
    oa_hbm = nc.dram_tensor("oa_fm", [D, NPAD], f16, kind=_dk)

    d_hN = nc.dram_tensor("d_hN", [NPAD, D], f16, kind="ExternalOutput") if dbg else None

    def fm3(handle):  # [D, NPAD] dram -> [128, KT, NPAD] AP view
        return handle.ap().rearrange("(k p) n -> p k n", p=128)

    with tile.TileContext(nc) as tc:
        import contextlib
        es = contextlib.ExitStack()
        with es:
            res = es.enter_context(tc.tile_pool(name="res", bufs=1))
            # resident loads
            h_res = res.tile([128, KT * NPAD], f16, tag="h")
            nc.sync.dma_start(h_res[:].rearrange("p (k n) -> p k n", k=KT), fm3(t_hfm))
            idx_sb = res.tile([128, C], i32, tag="idx")
            nc.sync.dma_start(idx_sb[:], t_idx.ap())
            dl_sb = res.tile([128, C], f16, tag="dl")
            nc.sync.dma_start(dl_sb[:], t_dl.ap())
            c16_sb = res.tile([128, 256], f16, tag="c16")
            nc.sync.dma_start(c16_sb[:], t_c16.ap())
            c32_sb = res.tile([128, 128 + NT + 20], f32, tag="c32")
            nc.sync.dma_start(c32_sb[:], t_c32.ap())

            iota_ap = c16_sb[:, 0:128]
            id16_ap = c16_sb[:, 128:256]
            id32_ap = c32_sb[:, 0:128]

            def invdeg(t):
                return c32_sb[:, 128 + t:128 + t + 1]

            def bias(bi, f):
                return c32_sb[:, 128 + NT + bi * FT + f:128 + NT + bi * FT + f + 1]

            wpool = es.enter_context(tc.tile_pool(name="wmat", bufs=8))
            wtiles = {}

            def load_w(widx):
                wt = wpool.tile([128, KT * 512], f16, tag="w")
                nc.sync.dma_start(wt[:], t_w.ap()[:, widx * KT * 512:(widx + 1) * KT * 512])
                wtiles[widx] = wt
                return wt

            def wap(widx, k, f):
                return wtiles[widx][:, k * 512 + f * 128:k * 512 + (f + 1) * 128]

            gpool = es.enter_context(tc.tile_pool(name="g", bufs=4))
            spool = es.enter_context(tc.tile_pool(name="s", bufs=4))
            nmpool = es.enter_context(tc.tile_pool(name="aggnm", bufs=2))
            stpool = es.enter_context(tc.tile_pool(name="stage", bufs=4))
            strpool = es.enter_context(tc.tile_pool(name="stream", bufs=1))
            tpool = es.enter_context(tc.tile_pool(name="tmp", bufs=2))

            def build_S(c):
                s = spool.tile([128, 128], f16, tag="s")
                nc.vector.tensor_tensor(
                    out=s[:], in0=dl_sb[:, c:c + 1].to_broadcast([128, 128]),
                    in1=iota_ap, op=ALU.is_equal)
                return s

            def gather(table_ap, c, width):
                g = gpool.tile([128, width], f16, tag="g")
                nc.gpsimd.indirect_dma_start(
                    out=g[:], out_offset=None, in_=table_ap,
                    in_offset=IndirectOffsetOnAxis(ap=idx_sb[:, c:c + 1], axis=0))
                return g

            # ============================================================
            # Phase A (+B pipelined): x_agg & h_agg ; hno, beta, hN in place
            # ============================================================
            for wi in (W_HID_WL, W_HID_WR, W_B1, W_B2):
                load_w(wi)
            with es.enter_context(contextlib.ExitStack()) as ab:
                hagg = ab.enter_context(tc.tile_pool(name="hagg", bufs=1)) \
                    .tile([128, KT * NPAD], f16, tag="hagg", name="hagg")
                psA = ab.enter_context(tc.tile_pool(name="psA", bufs=2, space="PSUM"))
                ptrA = ab.enter_context(tc.tile_pool(name="ptrA", bufs=2, space="PSUM"))
                psB = ab.enter_context(tc.tile_pool(name="psB", bufs=2, space="PSUM"))

                xa_stage = {}
                for t in range(NT):
                    ps = psA.tile([128, 1024], f32, tag="agg")
                    for j in range(M):
                        c = t * M + j
                        g = gather(t_xh.ap(), c, 1024)
                        s = build_S(c)
                        nc.tensor.matmul(ps[:, 0:512], lhsT=s[:], rhs=g[:, 0:512],
                                         start=(j == 0), stop=(j == M - 1))
                        nc.tensor.matmul(ps[:, 512:1024], lhsT=s[:], rhs=g[:, 512:1024],
                                         start=(j == 0), stop=(j == M - 1))
                    xa_nm = nmpool.tile([128, 512], f16, tag="nm")
                    nc.scalar.activation(out=xa_nm[:], in_=ps[:, 0:512], func=AF.Copy,
                                         bias=0.0, scale=invdeg(t))
                    ha_nm = nmpool.tile([128, 512], f16, tag="nm")
                    nc.scalar.activation(out=ha_nm[:], in_=ps[:, 512:1024], func=AF.Copy,
                                         bias=0.0, scale=invdeg(t))
                    # transpose to feature-major
                    if t % 4 == 0:
                        xa_stage = {f: stpool.tile([128, 512], f16, tag="xa",
                                                   name=f"xa{f}")
                                    for f in range(FT)}
                    for f in range(FT):
                        tp = ptrA.tile([128, 128], f16, tag="tr")
                        nc.tensor.transpose(out=tp[:], in_=xa_nm[:, f * 128:(f + 1) * 128],
                                            identity=id16_ap)
                        nc.vector.tensor_copy(
                            out=xa_stage[f][:, (t % 4) * 128:(t % 4 + 1) * 128], in_=tp[:])
                        tp2 = ptrA.tile([128, 128], f16, tag="tr")
                        nc.tensor.transpose(out=tp2[:], in_=ha_nm[:, f * 128:(f + 1) * 128],
                                            identity=id16_ap)
                        nc.vector.tensor_copy(
                            out=hagg[:, f * NPAD + t * 128:f * NPAD + (t + 1) * 128],
                            in_=tp2[:])
                    if t % 4 == 3 or t == NT - 1:
                        gw = (t % 4 + 1) * 128
                        g0 = (t // 4) * 512
                        for f in range(FT):
                            nc.sync.dma_start(
                                xagg_hbm.ap()[f * 128:(f + 1) * 128, g0:g0 + gw],
                                xa_stage[f][:, 0:gw])

                # ---- phase B: hno, beta, hN (in place over h_res)
                for c0, cw in _CHUNKS:
                    hno = {}
                    betas = {}
                    for f in range(FT):
                        ps = psB.tile([128, 512], f32, tag="mm")
                        step = 0
                        for wi, srct in ((W_HID_WL, hagg), (W_HID_WR, h_res)):
                            for k in range(KT):
                                nc.tensor.matmul(
                                    ps[:, 0:cw], lhsT=wap(wi, k, f),
                                    rhs=srct[:, k * NPAD + c0:k * NPAD + c0 + cw],
                                    start=(step == 0), stop=(step == 2 * KT - 1))
                                step += 1
                        hn = tpool.tile([128, 512], f16, tag="hno", bufs=5)
                        nc.scalar.activation(out=hn[:, 0:cw], in_=ps[:, 0:cw],
                                             func=AF.Identity, bias=bias(B_HNO, f))
                        hno[f] = hn
                    for f in range(FT):
                        ps = psB.tile([128, 512], f32, tag="mm")
                        step = 0
                        for k in range(KT):
                            nc.tensor.matmul(
                                ps[:, 0:cw], lhsT=wap(W_B1, k, f),
                                rhs=h_res[:, k * NPAD + c0:k * NPAD + c0 + cw],
                                start=(step == 0), stop=False)
                            step += 1
                        for k in range(KT):
                            step += 1
                            nc.tensor.matmul(
                                ps[:, 0:cw], lhsT=wap(W_B2, k, f),
                                rhs=hno[k][:, 0:cw],
                                start=False, stop=(step == 2 * KT))
                        beta = tpool.tile([128, 512], f16, tag="beta")
                        nc.scalar.activation(out=beta[:, 0:cw], in_=ps[:, 0:cw],
                                             func=AF.Tanh, bias=bias(B_BETA, f))
                        betas[f] = beta
                    # apply h += beta only after every beta of this chunk is
                    # computed -- the B2 matmuls above read all k-tiles of h
                    for f in range(FT):
                        hsl = h_res[:, f * NPAD + c0:f * NPAD + c0 + cw]
                        nc.vector.tensor_add(out=hsl, in0=hsl, in1=betas[f][:, 0:cw])

            # ============================================================
            # Phase C..E psum pools
            # ============================================================
            ptr = es.enter_context(tc.tile_pool(name="ptr", bufs=2, space="PSUM"))
            psagg = es.enter_context(tc.tile_pool(name="psagg", bufs=2, space="PSUM"))
            psmm = es.enter_context(tc.tile_pool(name="psmm", bufs=4, space="PSUM"))
            aggdp = es.enter_context(tc.tile_pool(name="aggd", bufs=1))

            # ---- phase C: hN -> node-major -> ag1in ; AllGather
            for t in range(NT):
                st = stpool.tile([128, 512], f16, tag="nmst", bufs=3)
                for f in range(FT):
                    tp = ptr.tile([128, 128], f16, tag="tr")
                    nc.tensor.transpose(
                        out=tp[:], in_=h_res[:, f * NPAD + t * 128:f * NPAD + (t + 1) * 128],
                        identity=id16_ap)
                    nc.vector.tensor_copy(out=st[:, f * 128:(f + 1) * 128], in_=tp[:])
                nc.sync.dma_start(ag1in.ap()[t * 128:(t + 1) * 128, :], st[:])
                if dbg:
                    nc.sync.dma_start(d_hN.ap()[t * 128:(t + 1) * 128, :], st[:])
            nc.gpsimd.collective_compute(
                "AllGather", ALU.bypass, replica_groups=RG,
                ins=[ag1in.ap()], outs=[ag1out.ap()])

            # ---- phase D1: hN_agg
            def agg_phase(table, dest_fm):
                for t in range(NT):
                    ps = psagg.tile([128, 512], f32, tag="agg5")
                    for j in range(M):
                        c = t * M + j
                        g = gather(table.ap(), c, 512)
                        s = build_S(c)
                        nc.tensor.matmul(ps[:, 0:512], lhsT=s[:], rhs=g[:],
                                         start=(j == 0), stop=(j == M - 1))
                    a_nm = nmpool.tile([128, 512], f16, tag="nm")
                    nc.scalar.activation(out=a_nm[:], in_=ps[:], func=AF.Copy,
                                         bias=0.0, scale=invdeg(t))
                    for f in range(FT):
                        tp = ptr.tile([128, 128], f16, tag="tr")
                        nc.tensor.transpose(out=tp[:], in_=a_nm[:, f * 128:(f + 1) * 128],
                                            identity=id16_ap)
                        nc.vector.tensor_copy(
                            out=dest_fm[:, f * NPAD + t * 128:f * NPAD + (t + 1) * 128],
                            in_=tp[:])

            hnagg = aggdp.tile([128, KT * NPAD], f16, tag="agg2")
            agg_phase(ag1out, hnagg)
            if dbg:
                t_hna = nc.dram_tensor("d_hnagg", [128, KT * NPAD], f16,
                                       kind="ExternalOutput")
                nc.sync.dma_start(t_hna.ap(), hnagg[:])

            # ---- phase D2: r, z, rh, outacc(=(z-1)*hN) ; rh -> ag2in
            for wi in (W_SX_WL, W_SX_WR, W_SH_WL, W_SH_WR,
                       W_UX_WL, W_UX_WR, W_UH_WL, W_UH_WR):
                load_w(wi)

            def stream_fm(hbm_handle, c0, cw, tag):
                xt = strpool.tile([128, KT * 512], f16, tag=tag)
                nc.sync.dma_start(
                    xt[:].rearrange("p (k n) -> p k n", k=KT)[:, :, 0:cw],
                    fm3(hbm_handle)[:, :, c0:c0 + cw])
                return xt

            for ci, (c0, cw) in enumerate(_CHUNKS):
                x_t = stream_fm(t_xfm, c0, cw, "xs")
                xa_t = stream_fm(xagg_hbm, c0, cw, "xas")
                nt0, ntn = c0 // 128, cw // 128
                rhnm = {nr: stpool.tile([128, 512], f16, tag="rhnm", name=f"rhnm{nr}")
                        for nr in range(ntn)}
                for f in range(FT):
                    srcs4 = ((W_SX_WL, xa_t, 512, 0), (W_SX_WR, x_t, 512, 0),
                             (W_SH_WL, hnagg, NPAD, c0), (W_SH_WR, h_res, NPAD, c0))
                    ps_r = psmm.tile([128, 512], f32, tag="mm")
                    step = 0
                    for wi, srct, stride, base in srcs4:
                        for k in range(KT):
                            nc.tensor.matmul(
                                ps_r[:, 0:cw], lhsT=wap(wi, k, f),
                                rhs=srct[:, k * stride + base:k * stride + base + cw],
                                start=(step == 0), stop=(step == 4 * KT - 1))
                            step += 1
                    r_t = tpool.tile([128, 512], f16, tag="r")
                    nc.scalar.activation(out=r_t[:, 0:cw], in_=ps_r[:, 0:cw],
                                         func=AF.Sigmoid, bias=bias(B_R, f))
                    srcs4z = ((W_UX_WL, xa_t, 512, 0), (W_UX_WR, x_t, 512, 0),
                              (W_UH_WL, hnagg, NPAD, c0), (W_UH_WR, h_res, NPAD, c0))
                    ps_z = psmm.tile([128, 512], f32, tag="mm")
                    step = 0
                    for wi, srct, stride, base in srcs4z:
                        for k in range(KT):
                            nc.tensor.matmul(
                                ps_z[:, 0:cw], lhsT=wap(wi, k, f),
                                rhs=srct[:, k * stride + base:k * stride + base + cw],
                                start=(step == 0), stop=(step == 4 * KT - 1))
                            step += 1
                    z_t = tpool.tile([128, 512], f16, tag="z")
                    nc.scalar.activation(out=z_t[:, 0:cw], in_=ps_z[:, 0:cw],
                                         func=AF.Sigmoid, bias=bias(B_Z, f))

                    hsl = h_res[:, f * NPAD + c0:f * NPAD + c0 + cw]
                    rh_t = tpool.tile([128, 512], f16, tag="rh")
                    nc.vector.tensor_mul(out=rh_t[:, 0:cw], in0=r_t[:, 0:cw], in1=hsl)
                    oa_t = tpool.tile([128, 512], f16, tag="oa")
                    nc.vector.scalar_tensor_tensor(
                        out=oa_t[:, 0:cw], in0=z_t[:, 0:cw], scalar=1.0, in1=hsl,
                        op0=ALU.subtract, op1=ALU.mult)
                    nc.sync.dma_start(rh_hbm.ap()[f * 128:(f + 1) * 128, c0:c0 + cw],
                                      rh_t[:, 0:cw])
                    nc.sync.dma_start(z_hbm.ap()[f * 128:(f + 1) * 128, c0:c0 + cw],
                                      z_t[:, 0:cw])
                    nc.sync.dma_start(oa_hbm.ap()[f * 128:(f + 1) * 128, c0:c0 + cw],
                                      oa_t[:, 0:cw])
                    for nr in range(ntn):
                        tp = ptr.tile([128, 128], f16, tag="tr")
                        nc.tensor.transpose(out=tp[:], in_=rh_t[:, nr * 128:(nr + 1) * 128],
                                            identity=id16_ap)
                        nc.vector.tensor_copy(out=rhnm[nr][:, f * 128:(f + 1) * 128],
                                              in_=tp[:])
                for nr in range(ntn):
                    nc.sync.dma_start(
                        ag2in.ap()[(nt0 + nr) * 128:(nt0 + nr + 1) * 128, :], rhnm[nr][:])

            nc.gpsimd.collective_compute(
                "AllGather", ALU.bypass, replica_groups=RG,
                ins=[ag2in.ap()], outs=[ag2out.ap()])

            # ---- phase E1: rh_agg
            rhagg = aggdp.tile([128, KT * NPAD], f16, tag="agg2")
            agg_phase(ag2out, rhagg)

            # ---- phase E2: h_tilde, out
            for wi in (W_CX_WL, W_CX_WR, W_CH_WL, W_CH_WR):
                load_w(wi)
            for c0, cw in _CHUNKS:
                x_t = stream_fm(t_xfm, c0, cw, "xs")
                xa_t = stream_fm(xagg_hbm, c0, cw, "xas")
                rh_t = stream_fm(rh_hbm, c0, cw, "rhs")
                nt0, ntn = c0 // 128, cw // 128
                out_f = {}
                for f in range(FT):
                    srcs4 = ((W_CX_WL, xa_t, 512, 0), (W_CX_WR, x_t, 512, 0),
                             (W_CH_WL, rhagg, NPAD, c0), (W_CH_WR, rh_t, 512, 0))
                    ps = psmm.tile([128, 512], f32, tag="mm")
                    step = 0
                    for wi, srct, stride, base in srcs4:
                        for k in range(KT):
                            nc.tensor.matmul(
                                ps[:, 0:cw], lhsT=wap(wi, k, f),
                                rhs=srct[:, k * stride + base:k * stride + base + cw],
                                start=(step == 0), stop=(step == 4 * KT - 1))
                            step += 1
                    ht_t = tpool.tile([128, 512], f16, tag="ht")
                    nc.scalar.activation(out=ht_t[:, 0:cw], in_=ps[:, 0:cw],
                                         func=AF.Tanh, bias=bias(B_HT, f))
                    z_t = tpool.tile([128, 512], f16, tag="z")
                    nc.sync.dma_start(z_t[:, 0:cw],
                                      z_hbm.ap()[f * 128:(f + 1) * 128, c0:c0 + cw])
                    oa_t = tpool.tile([128, 512], f16, tag="oa")
                    nc.sync.dma_start(oa_t[:, 0:cw],
                                      oa_hbm.ap()[f * 128:(f + 1) * 128, c0:c0 + cw])
                    t2 = tpool.tile([128, 512], f16, tag="t2")
                    nc.vector.tensor_mul(out=t2[:, 0:cw], in0=z_t[:, 0:cw],
                                         in1=ht_t[:, 0:cw])
                    of = tpool.tile([128, 512], f32, tag="outf", bufs=4)
                    # out = z*h_tilde - (z-1)*hN = (1-z)*hN + z*h_tilde
                    nc.vector.tensor_sub(out=of[:, 0:cw], in0=t2[:, 0:cw],
                                         in1=oa_t[:, 0:cw])
                    out_f[f] = of
                for nr in range(ntn):
                    st = stpool.tile([128, 512], f32, tag="outnm", bufs=3)
                    for f in range(FT):
                        tp = ptr.tile([128, 128], f32, tag="tr")
                        nc.tensor.transpose(out=tp[:],
                                            in_=out_f[f][:, nr * 128:(nr + 1) * 128],
                                            identity=id32_ap)
                        nc.vector.tensor_copy(out=st[:, f * 128:(f + 1) * 128], in_=tp[:])
                    nc.sync.dma_start(
                        t_out.ap()[(nt0 + nr) * 128:(nt0 + nr + 1) * 128, :], st[:])

    nc.compile()
    return nc


def _get_program(M):
    if M not in _PROGRAM_CACHE:
        _PROGRAM_CACHE[M] = _build_program(M)
    return _PROGRAM_CACHE[M]


def _in_maps(prep):
    maps = []
    for c in range(W):
        maps.append({
            "x_fm": np.ascontiguousarray(prep["x_fm"][c]),
            "h_fm": np.ascontiguousarray(prep["h_fm"][c]),
            "xh_tab": prep["xh_tab"],
            "idx_t": np.ascontiguousarray(prep["idx_T"][c]),
            "dstloc_t": np.ascontiguousarray(prep["dstloc_T"][c]),
            "wpack": prep["wpack"],
            "c16": prep["c16"],
            "c32": np.ascontiguousarray(prep["c32"][c]),
        })
    return maps


def _run(prep, trace=False):
    from concourse import bass_utils
    nc = _get_program(prep["M"])
    res = bass_utils.run_bass_kernel_spmd(
        nc, _in_maps(prep), core_ids=list(range(W)), trace=trace)
    out = np.concatenate([res.results[c]["out"][:NPC] for c in range(W)], axis=0)
    return np.ascontiguousarray(out.astype(np.float32)), res


def kernel(x, h, edge_index, params):
    prep = _host_prep(x, h, edge_index, params)
    out, _ = _run(prep, trace=False)
    return out


def benchmark(x, h, edge_index, params, iters=5):
    """Returns (output, exec_time_ns). Times device-resident repeated executes
    of the compiled NEFF across all 8 cores (min over iters)."""
    import time
    import jax
    from jax.sharding import Mesh, PartitionSpec
    from jax.experimental.shard_map import shard_map
    import concourse.mybir as mybir
    from concourse import bass2jax
    from concourse.bass2jax import _bass_exec_p, partition_id_tensor

    prep = _host_prep(x, h, edge_index, params)
    bass2jax.install_neuronx_cc_hook()
    nc = _get_program(prep["M"])

    in_names, out_names, out_avals = [], [], []
    pname = nc.partition_id_tensor.name if nc.partition_id_tensor else None
    for alloc in nc.m.functions[0].allocations:
        if not isinstance(alloc, mybir.MemoryLocationSet):
            continue
        name = alloc.memorylocations[0].name
        if alloc.kind == "ExternalInput":
            if name != pname:
                in_names.append(name)
        elif alloc.kind == "ExternalOutput":
            out_names.append(name)
            out_avals.append(jax.core.ShapedArray(
                tuple(alloc.tensor_shape), mybir.dt.np(alloc.dtype)))
    n_params = len(in_names)
    all_names = in_names + out_names
    if pname is not None:
        all_names = all_names + [pname]

    def _body(*args):
        operands = list(args)
        if pname is not None:
            operands.append(partition_id_tensor())
        return tuple(_bass_exec_p.bind(
            *operands, out_avals=tuple(out_avals), in_names=tuple(all_names),
            out_names=tuple(out_names), lowering_input_output_aliases=(),
            sim_require_finite=True, sim_require_nnan=True, nc=nc))

    devices = jax.devices()[:W]
    mesh = Mesh(np.asarray(devices), ("core",))
    nio = n_params + len(out_names)
    sharded = jax.jit(shard_map(
        _body, mesh=mesh, in_specs=(PartitionSpec("core"),) * nio,
        out_specs=(PartitionSpec("core"),) * len(out_names), check_rep=False),
        keep_unused=True)  # no donation -> inputs stay device-resident

    maps = _in_maps(prep)
    concat_in = [np.concatenate([np.asarray(maps[c][n]) for c in range(W)], axis=0)
                 for n in in_names]
    concat_zeros = [np.zeros((W * a.shape[0], *a.shape[1:]), a.dtype)
                    for a in out_avals]
    sharding = jax.sharding.NamedSharding(mesh, PartitionSpec("core"))
    dev_in = [jax.device_put(a, sharding) for a in concat_in]
    dev_zero = [jax.device_put(a, sharding) for a in concat_zeros]

    outs = sharded(*dev_in, *dev_zero)
    jax.block_until_ready(outs)
    oi = out_names.index("out")
    out_full = np.asarray(outs[oi]).reshape(W, NPAD, D)
    result = np.ascontiguousarray(
        out_full[:, :NPC, :].reshape(N, D).astype(np.float32))

    times = []
    for _ in range(iters):
        t0 = time.perf_counter()
        outs = sharded(*dev_in, *dev_zero)
        jax.block_until_ready(outs)
        times.append(time.perf_counter() - t0)
    return result, int(min(times) * 1e9)
